# revision 13
# baseline (speedup 1.0000x reference)
"""Trainium2 Bass kernel for nn_Attention (B=8, N=1024, C=768, 12 heads).

Sharding: pure data-parallel over batch — 8 cores, one batch element per
core, full weights replicated to every core. No collectives.

Per-core dataflow (tokens N=1024, channels C=768, heads 12 x 64):
  stage 1: qT/kT = w_qkv_tile.T @ xT   -> [1536, 1024] feature-major (f32r)
           v     = xT_tile.T @ w_v     -> token-major, stored as
                   v_aug [128, 65]-per-head bf16 tiles with a ones column
  stage 2: S^T[tk,tq] = kT_h.T @ qT_h  (fp32r, K=64 so full rate; head
           pairs run concurrently via row tiling: pairs live in
           partitions 0-63 / 64-127)
           attn_exp = exp(scale * S^T)             (ACT, PSUM -> bf16 SBUF)
           O^T_unnorm[65,tq] = v_aug.T @ attn_exp  (bf16; row 64 = softmax
           sums for free)
           normalize: broadcast sum row across 64 partitions with a K=1
           fp32r outer-product matmul, reciprocal_approx_fast on the
           broadcast tile, DVE multiply -> bf16 O^T
  stage 3: final[tq,:] = O^T_norm.T @ w_proj (bf16) + bias via K=1 fp32r
           ones-row matmul into the same PSUM accumulation, DMA out.

Inputs x/w_qkv/w_proj are pre-converted to bf16 on the host (x also
pre-transposed) — halves DMA and runs the tensor engine at 1 cycle/row.
"""

import os
import sys

import numpy as np

for _p in ("/opt/trn_rl_repo",):
    if os.path.isdir(_p) and _p not in sys.path:
        sys.path.append(_p)

import ml_dtypes

import concourse.bacc as bacc
import concourse.mybir as mybir
import concourse.tile as tile
from concourse.bass_utils import run_bass_kernel_spmd

F32 = mybir.dt.float32
F32R = mybir.dt.float32r
BF16 = mybir.dt.bfloat16
EXP = mybir.ActivationFunctionType.Exp

P = 128
B, N, C = 8, 1024, 768
NH, HD = 12, 64
C3 = 3 * C
KC = C // P          # 6 contraction tiles over channels
MQK = (2 * C) // P   # 12 output-feature tiles for q^T / k^T
NT = N // P          # 8 token tiles of 128
NT2 = N // 512       # 2 token slices of 512
VA = HD + 1          # 65: head dim + ones column
SCALE = float(HD) ** -0.5

_CACHE = {}


def _emit(nc, tc):
    xT_d = nc.dram_tensor("xT", [C, N], BF16, kind="ExternalInput")
    wqkv_d = nc.dram_tensor("w_qkv", [C, C3], BF16, kind="ExternalInput")
    wproj_d = nc.dram_tensor("w_proj", [C, C], BF16, kind="ExternalInput")
    bproj_d = nc.dram_tensor("b_proj", [1, C], F32R, kind="ExternalInput")
    out_d = nc.dram_tensor("out", [N, C], F32, kind="ExternalOutput")

    mm = nc.tensor.matmul
    VP = P  # padded per-head stride in v_aug (head dim + ones col + pad)

    from contextlib import ExitStack

    with ExitStack() as es:
        const = es.enter_context(tc.tile_pool(name="const", bufs=1))
        big = es.enter_context(tc.tile_pool(name="big", bufs=1))

        onesf = const.tile([1, P], F32, tag="onesf", name="onesf")
        nc.vector.memset(onesf[:], 1.0)
        ones = const.tile([1, P], F32R, tag="ones", name="ones")
        nc.vector.tensor_copy(ones[:], onesf[:])
        onesc = const.tile([P, NH], F32, tag="onesc", name="onesc")
        nc.vector.memset(onesc[:], 1.0)
        bproj_sb = const.tile([1, C], F32R, tag="bproj", name="bproj")
        nc.sync.dma_start(bproj_sb[:], bproj_d.ap())

        xT = [big.tile([P, N], BF16, tag=f"xT{k}", name=f"xT{k}")
              for k in range(KC)]
        wq = [big.tile([P, C3], BF16, tag=f"wq{k}", name=f"wq{k}")
              for k in range(KC)]
        qkT = [big.tile([P, N], BF16, tag=f"qk{m}", name=f"qk{m}")
               for m in range(MQK)]
        vaug = [big.tile([P, NH * VP], BF16, tag=f"va{t}", name=f"va{t}")
                for t in range(NT)]
        oT = [big.tile([P, N], BF16, tag=f"oT{i}", name=f"oT{i}")
              for i in range(KC)]
        wproj_sb = [big.tile([P, C], BF16, tag=f"wp{k}", name=f"wp{k}")
                    for k in range(KC)]

        with tc.tile_pool(name="warmp", bufs=1) as warmp, \
             tc.tile_pool(name="ps1", bufs=4, space="PSUM") as ps1:
            # Warm the PE clock (HAM) with dummy matmuls while DMAs land.
            warm_sb = warmp.tile([P, 512], BF16, tag="warm", name="warm")
            nc.vector.memset(warm_sb[:], 0.0)
            warm_ps = ps1.tile([P, 512], F32, tag="warmps", name="warmps")
            for _ in range(24):
                mm(warm_ps[:], warm_sb[:, 0:P], warm_sb[:],
                   start=True, stop=True)

            for k in range(KC):
                nc.sync.dma_start(xT[k][:], xT_d.ap()[k * P:(k + 1) * P, :])
                nc.sync.dma_start(wq[k][:, 2 * C:],
                                  wqkv_d.ap()[k * P:(k + 1) * P, 2 * C:])
            for k in range(KC):
                nc.sync.dma_start(wq[k][:, 0:2 * C],
                                  wqkv_d.ap()[k * P:(k + 1) * P, 0:2 * C])
            for k in range(KC):
                nc.sync.dma_start(wproj_sb[k][:],
                                  wproj_d.ap()[k * P:(k + 1) * P, :])

            # v: token-major [tokens 128, feat], scattered into v_aug tiles
            # (per-head stride VP=128: cols 0:64 data, 64 ones, 65:128 pad)
            for t in range(NT):
                nc.gpsimd.memset(vaug[t][:], 0.0)
                for (n0, nw) in ((0, 512), (512, 256)):
                    ps = ps1.tile([P, 512], F32, tag="ps", name="ps")
                    for k in range(KC):
                        mm(ps[:, 0:nw], xT[k][:, t * P:(t + 1) * P],
                           wq[k][:, 2 * C + n0: 2 * C + n0 + nw],
                           start=(k == 0), stop=(k == KC - 1))
                    h0, hn = n0 // HD, nw // HD
                    dst = vaug[t][:].rearrange("p (h m) -> p h m", m=VP)
                    src = ps[:, 0:nw].rearrange("p (h m) -> p h m", m=HD)
                    nc.vector.tensor_copy(dst[:, h0:h0 + hn, 0:HD], src)
                nc.vector.tensor_copy(
                    vaug[t][:].rearrange("p (h m) -> p h m", m=VP)[:, :, HD:HD + 1],
                    onesc[:].rearrange("p (h o) -> p h o", o=1))

        # ------- merged stage 1 (q^T/k^T) + stage 2 (attention) -------
        with tc.tile_pool(name="attn", bufs=6) as attn_pool, \
             tc.tile_pool(name="small", bufs=3) as small, \
             tc.tile_pool(name="ps_s", bufs=2, space="PSUM") as ps_s, \
             tc.tile_pool(name="ps_pv", bufs=2, space="PSUM") as ps_pv, \
             tc.tile_pool(name="ps_qk", bufs=1, space="PSUM") as ps_qk, \
             tc.tile_pool(name="ps_bc", bufs=1, space="PSUM") as ps_bc:
            for hp in range(NH // 2):
                # q^T / k^T tiles for this head pair, feature-major
                for m in (hp, NH // 2 + hp):
                    for n2 in range(NT2):
                        ps = ps_qk.tile([P, 512], F32, tag="ps", name="ps")
                        for k in range(KC):
                            mm(ps[:], wq[k][:, m * P:(m + 1) * P],
                               xT[k][:, n2 * 512:(n2 + 1) * 512],
                               start=(k == 0), stop=(k == KC - 1))
                        nc.vector.tensor_copy(
                            qkT[m][:, n2 * 512:(n2 + 1) * 512], ps[:])

                qt, kt = qkT[hp], qkT[NH // 2 + hp]
                for n2 in range(NT2):
                    tq = slice(n2 * 512, (n2 + 1) * 512)
                    pv = [ps_pv.tile([P, 512], F32, tag="pv", name="pv")
                          for _ in range(2)]

                    def s_pair(j):
                        sA = ps_s.tile([P, 1024], F32, tag="s", name="s")
                        sB = ps_s.tile([P, 1024], F32, tag="s", name="s")
                        for u in range(2):
                            mk = 2 * j + u
                            us = slice(u * 512, (u + 1) * 512)
                            t0 = slice(mk * P, mk * P + HD)
                            t1 = slice(mk * P + HD, (mk + 1) * P)
                            mm(sA[0:HD, us], kt[0:HD, t0], qt[0:HD, tq],
                               start=True, stop=True, tile_position=(0, 0))
                            mm(sA[HD:P, us], kt[0:HD, t1], qt[0:HD, tq],
                               start=True, stop=True, tile_position=(0, HD))
                            mm(sB[0:HD, us], kt[HD:P, t0], qt[HD:P, tq],
                               start=True, stop=True, tile_position=(HD, 0))
                            mm(sB[HD:P, us], kt[HD:P, t1], qt[HD:P, tq],
                               start=True, stop=True, tile_position=(HD, HD))
                        return sA, sB

                    s_cur = s_pair(0)
                    for j in range(4):
                        e = []
                        for half, s_ps in enumerate(s_cur):
                            et = attn_pool.tile([P, 1024], BF16, tag="e",
                                                name="e")
                            nc.scalar.activation(et[:], s_ps[:], EXP,
                                                 scale=SCALE)
                            e.append(et)
                        if j + 1 < 4:
                            s_cur = s_pair(j + 1)
                        for u in range(2):
                            mk = 2 * j + u
                            for half in range(2):
                                h = 2 * hp + half
                                mm(pv[half][:],
                                   vaug[mk][:, h * VP:(h + 1) * VP],
                                   e[half][:, u * 512:(u + 1) * 512],
                                   start=(mk == 0), stop=(mk == NT - 1))

                    bcs = []
                    for half in range(2):
                        sums = small.tile([1, 512], F32R, tag="sums",
                                          name="sums")
                        nc.vector.tensor_copy(sums[:], pv[half][HD:VA, :])
                        bc = ps_bc.tile([HD, 512], F32, tag="bc", name="bc")
                        mm(bc[:], ones[0:1, 0:HD], sums[:],
                           start=True, stop=True)
                        bcs.append(bc)
                    for half in range(2):
                        bc_sb = small.tile([HD, 512], F32, tag="bcs",
                                           name="bcs")
                        nc.vector.reciprocal_approx_fast(bc_sb[:],
                                                         bcs[half][:])
                        nc.vector.tensor_mul(
                            oT[hp][half * HD:(half + 1) * HD, tq],
                            pv[half][0:HD, :], bc_sb[:])

        # ---------------- stage 3: output projection ----------------
        with tc.tile_pool(name="outp", bufs=3) as out_pool, \
             tc.tile_pool(name="warm3", bufs=1) as warmp3, \
             tc.tile_pool(name="ps3a", bufs=2, space="PSUM") as ps3a, \
             tc.tile_pool(name="ps3b", bufs=2, space="PSUM") as ps3b:
            warm3_sb = warmp3.tile([P, 512], BF16, tag="warm3", name="warm3")
            nc.vector.memset(warm3_sb[:], 0.0)
            warm3_ps = ps3a.tile([P, 512], F32, tag="warm3ps", name="warm3ps")
            for _ in range(16):
                mm(warm3_ps[:], warm3_sb[:, 0:P], warm3_sb[:],
                   start=True, stop=True)
            for t in range(NT):
                tq = slice(t * P, (t + 1) * P)
                psa = ps3a.tile([P, 512], F32, tag="fa", name="fa")
                psb = ps3b.tile([P, 256], F32, tag="fb", name="fb")
                for k in range(KC):
                    mm(psa[:], oT[k][:, tq], wproj_sb[k][:, 0:512],
                       start=(k == 0), stop=False)
                for k in range(KC):
                    mm(psb[:], oT[k][:, tq], wproj_sb[k][:, 512:768],
                       start=(k == 0), stop=False)
                mm(psa[:], ones[0:1, :], bproj_sb[0:1, 0:512],
                   start=False, stop=True)
                mm(psb[:], ones[0:1, :], bproj_sb[0:1, 512:768],
                   start=False, stop=True)
                ot = out_pool.tile([P, C], F32, tag="out", name="outt")
                nc.vector.tensor_copy(ot[:, 0:512], psa[:])
                nc.vector.tensor_copy(ot[:, 512:768], psb[:])
                nc.sync.dma_start(out_d.ap()[tq, :], ot[:])


def build():
    if "nc" in _CACHE:
        return _CACHE["nc"]
    nc = bacc.Bacc("TRN2", target_bir_lowering=False, debug=False)
    with tile.TileContext(nc) as tc:
        _emit(nc, tc)
    nc.compile()
    _CACHE["nc"] = nc
    return nc


def make_in_maps(x, w_qkv, w_proj, b_proj):
    x = np.asarray(x, dtype=np.float32)
    w_qkv = np.asarray(w_qkv, dtype=np.float32).astype(ml_dtypes.bfloat16)
    w_proj = np.asarray(w_proj, dtype=np.float32).astype(ml_dtypes.bfloat16)
    b_proj = np.ascontiguousarray(
        np.asarray(b_proj, dtype=np.float32).reshape(1, C))
    return [
        {
            "xT": np.ascontiguousarray(x[i].T.astype(ml_dtypes.bfloat16)),
            "w_qkv": w_qkv,
            "w_proj": w_proj,
            "b_proj": b_proj,
        }
        for i in range(B)
    ]


def run(x, w_qkv, w_proj, b_proj, **spmd_kwargs):
    nc = build()
    in_maps = make_in_maps(x, w_qkv, w_proj, b_proj)
    res = run_bass_kernel_spmd(nc, in_maps, core_ids=list(range(B)),
                               **spmd_kwargs)
    out = np.stack([res.results[i]["out"] for i in range(B)])
    return out.astype(np.float32), res


def kernel(x, w_qkv, w_proj, b_proj, H=None, W=None, **_ignored):
    out, _ = run(x, w_qkv, w_proj, b_proj)
    return out


# revision 14
# speedup vs baseline: 1.0893x; 1.0893x over previous
"""Trainium2 Bass kernel for nn_Attention (B=8, N=1024, C=768, 12 heads).

Sharding: pure data-parallel over batch — 8 cores, one batch element per
core, full weights replicated to every core. No collectives.

Per-core dataflow (tokens N=1024, channels C=768, heads 12 x 64):
  stage 1: qT/kT = w_qkv_tile.T @ xT   -> [1536, 1024] feature-major (f32r)
           v     = xT_tile.T @ w_v     -> token-major, stored as
                   v_aug [128, 65]-per-head bf16 tiles with a ones column
  stage 2: S^T[tk,tq] = kT_h.T @ qT_h  (fp32r, K=64 so full rate; head
           pairs run concurrently via row tiling: pairs live in
           partitions 0-63 / 64-127)
           attn_exp = exp(scale * S^T)             (ACT, PSUM -> bf16 SBUF)
           O^T_unnorm[65,tq] = v_aug.T @ attn_exp  (bf16; row 64 = softmax
           sums for free)
           normalize: broadcast sum row across 64 partitions with a K=1
           fp32r outer-product matmul, reciprocal_approx_fast on the
           broadcast tile, DVE multiply -> bf16 O^T
  stage 3: final[tq,:] = O^T_norm.T @ w_proj (bf16) + bias via K=1 fp32r
           ones-row matmul into the same PSUM accumulation, DMA out.

Inputs x/w_qkv/w_proj are pre-converted to bf16 on the host (x also
pre-transposed) — halves DMA and runs the tensor engine at 1 cycle/row.
"""

import os
import sys

import numpy as np

for _p in ("/opt/trn_rl_repo",):
    if os.path.isdir(_p) and _p not in sys.path:
        sys.path.append(_p)

import ml_dtypes

import concourse.bacc as bacc
import concourse.mybir as mybir
import concourse.tile as tile
from concourse.bass_utils import run_bass_kernel_spmd

F32 = mybir.dt.float32
F32R = mybir.dt.float32r
BF16 = mybir.dt.bfloat16
EXP = mybir.ActivationFunctionType.Exp

P = 128
B, N, C = 8, 1024, 768
NH, HD = 12, 64
C3 = 3 * C
KC = C // P          # 6 contraction tiles over channels
MQK = (2 * C) // P   # 12 output-feature tiles for q^T / k^T
NT = N // P          # 8 token tiles of 128
NT2 = N // 512       # 2 token slices of 512
VA = HD + 1          # 65: head dim + ones column
SCALE = float(HD) ** -0.5

_CACHE = {}


def _emit(nc, tc):
    xT_d = nc.dram_tensor("xT", [C, N], BF16, kind="ExternalInput")
    wqkv_d = nc.dram_tensor("w_qkv", [C, C3], BF16, kind="ExternalInput")
    wproj_d = nc.dram_tensor("w_proj", [C, C], BF16, kind="ExternalInput")
    bproj_d = nc.dram_tensor("b_proj", [1, C], F32R, kind="ExternalInput")
    out_d = nc.dram_tensor("out", [N, C], F32, kind="ExternalOutput")

    mm = nc.tensor.matmul
    VP = P  # padded per-head stride in v_aug (head dim + ones col + pad)

    from contextlib import ExitStack

    with ExitStack() as es:
        const = es.enter_context(tc.tile_pool(name="const", bufs=1))
        big = es.enter_context(tc.tile_pool(name="big", bufs=1))

        onesf = const.tile([1, P], F32, tag="onesf", name="onesf")
        nc.vector.memset(onesf[:], 1.0)
        ones = const.tile([1, P], F32R, tag="ones", name="ones")
        nc.vector.tensor_copy(ones[:], onesf[:])
        onesc = const.tile([P, NH], F32, tag="onesc", name="onesc")
        nc.vector.memset(onesc[:], 1.0)
        bproj_sb = const.tile([1, C], F32R, tag="bproj", name="bproj")
        nc.sync.dma_start(bproj_sb[:], bproj_d.ap())

        xT = [big.tile([P, N], BF16, tag=f"xT{k}", name=f"xT{k}")
              for k in range(KC)]
        wq = [big.tile([P, C3], BF16, tag=f"wq{k}", name=f"wq{k}")
              for k in range(KC)]
        qkT = [big.tile([P, N], BF16, tag=f"qk{m}", name=f"qk{m}")
               for m in range(MQK)]
        vaug = [big.tile([P, NH * VP], BF16, tag=f"va{t}", name=f"va{t}")
                for t in range(NT)]
        oT = [big.tile([P, N], BF16, tag=f"oT{i}", name=f"oT{i}")
              for i in range(KC)]
        wproj_sb = [big.tile([P, C], BF16, tag=f"wp{k}", name=f"wp{k}")
                    for k in range(KC)]

        with tc.tile_pool(name="warmp", bufs=1) as warmp, \
             tc.tile_pool(name="ps1", bufs=4, space="PSUM") as ps1:
            # Warm the PE clock (HAM) with dummy matmuls while DMAs land.
            warm_sb = warmp.tile([P, 512], BF16, tag="warm", name="warm")
            nc.vector.memset(warm_sb[:], 0.0)
            warm_ps = ps1.tile([P, 512], F32, tag="warmps", name="warmps")
            for _ in range(24):
                mm(warm_ps[:], warm_sb[:, 0:P], warm_sb[:],
                   start=True, stop=True)

            for k in range(KC):
                nc.sync.dma_start(xT[k][:], xT_d.ap()[k * P:(k + 1) * P, :])
                nc.sync.dma_start(wq[k][:, 2 * C:],
                                  wqkv_d.ap()[k * P:(k + 1) * P, 2 * C:])
            for k in range(KC):
                nc.sync.dma_start(wq[k][:, 0:2 * C],
                                  wqkv_d.ap()[k * P:(k + 1) * P, 0:2 * C])
            for k in range(KC):
                nc.sync.dma_start(wproj_sb[k][:],
                                  wproj_d.ap()[k * P:(k + 1) * P, :])

            # v: token-major [tokens 128, feat], scattered into v_aug tiles
            # (per-head stride VP=128: cols 0:64 data, 64 ones, 65:128 pad)
            for t in range(NT):
                nc.gpsimd.memset(vaug[t][:], 0.0)
                for (n0, nw) in ((0, 512), (512, 256)):
                    ps = ps1.tile([P, 512], F32, tag="ps", name="ps")
                    for k in range(KC):
                        mm(ps[:, 0:nw], xT[k][:, t * P:(t + 1) * P],
                           wq[k][:, 2 * C + n0: 2 * C + n0 + nw],
                           start=(k == 0), stop=(k == KC - 1))
                    h0, hn = n0 // HD, nw // HD
                    dst = vaug[t][:].rearrange("p (h m) -> p h m", m=VP)
                    src = ps[:, 0:nw].rearrange("p (h m) -> p h m", m=HD)
                    nc.vector.tensor_copy(dst[:, h0:h0 + hn, 0:HD], src)
                nc.vector.tensor_copy(
                    vaug[t][:].rearrange("p (h m) -> p h m", m=VP)[:, :, HD:HD + 1],
                    onesc[:].rearrange("p (h o) -> p h o", o=1))

        # ------- merged stage 1 (q^T/k^T) + stage 2 (attention) -------
        with tc.tile_pool(name="attn", bufs=6) as attn_pool, \
             tc.tile_pool(name="small", bufs=3) as small, \
             tc.tile_pool(name="ps_s", bufs=2, space="PSUM") as ps_s, \
             tc.tile_pool(name="ps_pv", bufs=2, space="PSUM") as ps_pv, \
             tc.tile_pool(name="ps_qk", bufs=1, space="PSUM") as ps_qk, \
             tc.tile_pool(name="ps_bc", bufs=1, space="PSUM") as ps_bc:
            for hp in range(NH // 2):
                # q^T / k^T tiles for this head pair, feature-major
                for m in (hp, NH // 2 + hp):
                    for n2 in range(NT2):
                        ps = ps_qk.tile([P, 512], F32, tag="ps", name="ps")
                        for k in range(KC):
                            mm(ps[:], wq[k][:, m * P:(m + 1) * P],
                               xT[k][:, n2 * 512:(n2 + 1) * 512],
                               start=(k == 0), stop=(k == KC - 1))
                        nc.vector.tensor_copy(
                            qkT[m][:, n2 * 512:(n2 + 1) * 512], ps[:])

                qt, kt = qkT[hp], qkT[NH // 2 + hp]
                for n2 in range(NT2):
                    tq = slice(n2 * 512, (n2 + 1) * 512)
                    pv = [ps_pv.tile([P, 512], F32, tag="pv", name="pv")
                          for _ in range(2)]

                    def s_pair(j):
                        sA = ps_s.tile([P, 1024], F32, tag="s", name="s")
                        sB = ps_s.tile([P, 1024], F32, tag="s", name="s")
                        for u in range(2):
                            mk = 2 * j + u
                            tkc = slice(mk * P, (mk + 1) * P)
                            mm(sA[:, u * 512:(u + 1) * 512], kt[0:64, tkc],
                               qt[0:64, tq], start=True, stop=True)
                            mm(sB[:, u * 512:(u + 1) * 512], kt[64:128, tkc],
                               qt[64:128, tq], start=True, stop=True)
                        return sA, sB

                    s_cur = s_pair(0)
                    for j in range(4):
                        e = []
                        for half, s_ps in enumerate(s_cur):
                            et = attn_pool.tile([P, 1024], BF16, tag="e",
                                                name="e")
                            nc.scalar.activation(et[:], s_ps[:], EXP,
                                                 scale=SCALE)
                            e.append(et)
                        if j + 1 < 4:
                            s_cur = s_pair(j + 1)
                        for u in range(2):
                            mk = 2 * j + u
                            for half in range(2):
                                h = 2 * hp + half
                                mm(pv[half][:],
                                   vaug[mk][:, h * VP:(h + 1) * VP],
                                   e[half][:, u * 512:(u + 1) * 512],
                                   start=(mk == 0), stop=(mk == NT - 1))

                    bcs = []
                    for half in range(2):
                        sums = small.tile([1, 512], F32R, tag="sums",
                                          name="sums")
                        nc.vector.tensor_copy(sums[:], pv[half][HD:VA, :])
                        bc = ps_bc.tile([HD, 512], F32, tag="bc", name="bc")
                        mm(bc[:], ones[0:1, 0:HD], sums[:],
                           start=True, stop=True)
                        bcs.append(bc)
                    for half in range(2):
                        bc_sb = small.tile([HD, 512], F32, tag="bcs",
                                           name="bcs")
                        nc.vector.reciprocal_approx_fast(bc_sb[:],
                                                         bcs[half][:])
                        nc.vector.tensor_mul(
                            oT[hp][half * HD:(half + 1) * HD, tq],
                            pv[half][0:HD, :], bc_sb[:])

        # ---------------- stage 3: output projection ----------------
        with tc.tile_pool(name="outp", bufs=3) as out_pool, \
             tc.tile_pool(name="warm3", bufs=1) as warmp3, \
             tc.tile_pool(name="ps3a", bufs=2, space="PSUM") as ps3a, \
             tc.tile_pool(name="ps3b", bufs=2, space="PSUM") as ps3b:
            warm3_sb = warmp3.tile([P, 512], BF16, tag="warm3", name="warm3")
            nc.vector.memset(warm3_sb[:], 0.0)
            warm3_ps = ps3a.tile([P, 512], F32, tag="warm3ps", name="warm3ps")
            for _ in range(16):
                mm(warm3_ps[:], warm3_sb[:, 0:P], warm3_sb[:],
                   start=True, stop=True)
            for t in range(NT):
                tq = slice(t * P, (t + 1) * P)
                psa = ps3a.tile([P, 512], F32, tag="fa", name="fa")
                psb = ps3b.tile([P, 256], F32, tag="fb", name="fb")
                for k in range(KC):
                    mm(psa[:], oT[k][:, tq], wproj_sb[k][:, 0:512],
                       start=(k == 0), stop=False)
                for k in range(KC):
                    mm(psb[:], oT[k][:, tq], wproj_sb[k][:, 512:768],
                       start=(k == 0), stop=False)
                mm(psa[:], ones[0:1, :], bproj_sb[0:1, 0:512],
                   start=False, stop=True)
                mm(psb[:], ones[0:1, :], bproj_sb[0:1, 512:768],
                   start=False, stop=True)
                ot = out_pool.tile([P, C], F32, tag="out", name="outt")
                nc.vector.tensor_copy(ot[:, 0:512], psa[:])
                nc.vector.tensor_copy(ot[:, 512:768], psb[:])
                nc.sync.dma_start(out_d.ap()[tq, :], ot[:])


def build():
    if "nc" in _CACHE:
        return _CACHE["nc"]
    nc = bacc.Bacc("TRN2", target_bir_lowering=False, debug=False)
    with tile.TileContext(nc) as tc:
        _emit(nc, tc)
    nc.compile()
    _CACHE["nc"] = nc
    return nc


def make_in_maps(x, w_qkv, w_proj, b_proj):
    x = np.asarray(x, dtype=np.float32)
    w_qkv = np.asarray(w_qkv, dtype=np.float32).astype(ml_dtypes.bfloat16)
    w_proj = np.asarray(w_proj, dtype=np.float32).astype(ml_dtypes.bfloat16)
    b_proj = np.ascontiguousarray(
        np.asarray(b_proj, dtype=np.float32).reshape(1, C))
    return [
        {
            "xT": np.ascontiguousarray(x[i].T.astype(ml_dtypes.bfloat16)),
            "w_qkv": w_qkv,
            "w_proj": w_proj,
            "b_proj": b_proj,
        }
        for i in range(B)
    ]


def run(x, w_qkv, w_proj, b_proj, **spmd_kwargs):
    nc = build()
    in_maps = make_in_maps(x, w_qkv, w_proj, b_proj)
    res = run_bass_kernel_spmd(nc, in_maps, core_ids=list(range(B)),
                               **spmd_kwargs)
    out = np.stack([res.results[i]["out"] for i in range(B)])
    return out.astype(np.float32), res


def kernel(x, w_qkv, w_proj, b_proj, H=None, W=None, **_ignored):
    out, _ = run(x, w_qkv, w_proj, b_proj)
    return out
